# revision 1
# baseline (speedup 1.0000x reference)
"""Trainium2 Bass kernel for nn_NUFFTLayerMultiChannelInitMixed.

Math: the reference's spread->FFT->filter->IFFT->energy pipeline is an exact
bilinear form in the (analytic) spectrum of the periodized Gaussians:

  ghat_n(k) = Cc * sum_{r in -1,0,1} exp(-tau (k+rM)^2) exp(-i (k+rM) x_n)

With alpha_n = cos(M x_n), beta_n = sin(M x_n), A = p_k + q_k alpha_n,
B = d_k beta_n (p/q/d from the alias weights), c = cos(k x_n), s = sin(k x_n):

  Re ghat = A c - B s,   -Im ghat = A s + B c,   |ghat|^2 = A^2 + B^2  (!)

Energy per channel i (filter F_i(k) = deconv^2 * mult_i, even in k):

  e_i[n] = pref * ( T_i[n] - self_i[n] )
  T_i[n] = sum_k w_k F_i (Re_n ReS + Im_n ImS)   (S = sum over points)
  self_i[n] = quadratic polynomial in alpha_n, beta_n (no trig!)

T_i reduces to two small matmul families against the [K, N] cos/sin
matrices (K = 128 truncation keeps rel err ~2e-5; the filter decays ~1/k^2).
Sharding: batch-parallel, 2 of 16 batches per core, no collectives.
"""

import numpy as np

try:
    import concourse.bass as bass
except ImportError:
    import sys
    sys.path.insert(0, "/opt/trn_rl_repo")
    import concourse.bass as bass

import concourse.bacc as bacc
import concourse.mybir as mybir
from concourse import tile
from concourse.bass_utils import run_bass_kernel_spmd

F32 = mybir.dt.float32
AF = mybir.ActivationFunctionType
ALU = mybir.AluOpType

M = 2001
L = 2.0 * np.pi
TAU = 12.0 * (L / (2.0 * np.pi * M)) ** 2
KTRUNC = 128
B_FULL, N = 16, 1024
NCORES = 8
BPC = B_FULL // NCORES  # batches per core
MAGIC = 12582912.0      # 1.5 * 2^23: (u + MAGIC) - MAGIC = round-to-nearest(u)
PI = float(np.pi)


def _host_constants(shift0, shift1, amp0, amp1):
    """fp64 host-side k-space constants -> packed [128, 12] table + scalars."""
    k = np.arange(KTRUNC, dtype=np.float64)
    tau = float(TAU)
    p = np.exp(-tau * k * k)
    apl = np.exp(-tau * (k + M) ** 2)
    amn = np.exp(-tau * (k - M) ** 2)
    q = apl + amn
    d = apl - amn
    Cc = (M / L) * np.sqrt(4.0 * np.pi * tau)
    deconv2 = (np.pi / tau) * np.exp(2.0 * tau * k * k)
    mult1 = float(amp0) * (4.0 * np.pi) / (k * k + (1.0 * float(shift0)) ** 2)
    mult2 = float(amp1) * (4.0 * np.pi) / (k * k + (0.5 * float(shift1)) ** 2)
    w = np.full(KTRUNC, 2.0)
    w[0] = 1.0
    scale = 1.0 / ((2.0 * np.pi * M / L) * (2.0 * np.pi))
    pref = scale * Cc * Cc / M
    wF1 = w * deconv2 * mult1
    wF2 = w * deconv2 * mult2

    cst = np.zeros((128, 12), dtype=np.float64)
    cst[:, 0] = p
    cst[:, 1] = q
    cst[:, 2] = -d          # dneg (for S_R)
    cst[:, 3] = d           # (for S_I)
    cst[:, 4] = pref * p * wF1
    cst[:, 5] = pref * q * wF1
    cst[:, 6] = pref * p * wF2
    cst[:, 7] = pref * q * wF2
    cst[:, 8] = pref * d * wF1
    cst[:, 9] = pref * d * wF2
    cst[:, 10] = -pref * d * wF1
    cst[:, 11] = -pref * d * wF2

    def selfsc(wF):
        return [float(pref * np.sum(wF * p * p)),
                float(pref * 2.0 * np.sum(wF * p * q)),
                float(pref * np.sum(wF * q * q)),
                float(pref * np.sum(wF * d * d))]

    return cst.astype(np.float32), selfsc(wF1), selfsc(wF2)


def _emit_batch(nc, tc, pools, b, t_in, out_t, kv, cst, ident, sc1, sc2):
    pc, wp, sp, ps_u, ps_t, ps_cs, ps_T = pools
    KT = KTRUNC

    # --- phase matrix u = k (x) t  (outer product on PE), then r = u - rni(u)
    trow = sp.tile([1, N], F32, tag="trow")
    nc.sync.dma_start(trow[:], t_in[b])
    t88 = sp.tile([128, 8], F32, tag="t88")
    nc.sync.dma_start(t88[:], t_in[b].rearrange("(j p) -> p j", p=128))

    u_ps = ps_u.tile([128, N], F32, tag="u")
    nc.tensor.matmul(u_ps[:, 0:512], kv[:], trow[:, 0:512], start=True, stop=True)
    nc.tensor.matmul(u_ps[:, 512:1024], kv[:], trow[:, 512:1024], start=True, stop=True)

    rni = wp.tile([128, N], F32, tag="rni")
    nc.vector.tensor_scalar(rni[:], u_ps[:], MAGIC, MAGIC, ALU.add, ALU.subtract)
    r = wp.tile([128, N], F32, tag="r")
    nc.vector.tensor_sub(r[:], u_ps[:], rni[:])

    # --- big trig matrices (layout [k-part, n-free]); true cos/sin values
    smat = wp.tile([128, N], F32, tag="smat")
    nc.scalar.activation(smat[:], r[:], AF.Sin, scale=2.0 * PI)
    h = wp.tile([128, N], F32, tag="h")
    nc.scalar.activation(h[:], r[:], AF.Sin, scale=PI)
    hh = wp.tile([128, N], F32, tag="hh")
    nc.vector.tensor_mul(hh[:], h[:], h[:])
    cmat = wp.tile([128, N], F32, tag="cmat")
    nc.vector.tensor_scalar(cmat[:], hh[:], 2.0, 1.0, ALU.mult, ALU.subtract)

    # --- alpha/beta = cos/sin(M x) in [128, 8] (n = 128*j + p)
    u8 = sp.tile([128, 8], F32, tag="u8")
    nc.vector.tensor_scalar_mul(u8[:], t88[:], float(M))
    rni8 = sp.tile([128, 8], F32, tag="rni8")
    nc.vector.tensor_scalar(rni8[:], u8[:], MAGIC, MAGIC, ALU.add, ALU.subtract)
    r8 = sp.tile([128, 8], F32, tag="r8")
    nc.vector.tensor_sub(r8[:], u8[:], rni8[:])
    beta = sp.tile([128, 8], F32, tag="beta")
    nc.scalar.activation(beta[:], r8[:], AF.Sin, scale=2.0 * PI)
    h8 = sp.tile([128, 8], F32, tag="h8")
    nc.scalar.activation(h8[:], r8[:], AF.Sin, scale=PI)
    hh8 = sp.tile([128, 8], F32, tag="hh8")
    nc.vector.tensor_mul(hh8[:], h8[:], h8[:])
    alpha = sp.tile([128, 8], F32, tag="alpha")
    nc.vector.tensor_scalar(alpha[:], hh8[:], 2.0, 1.0, ALU.mult, ALU.subtract)

    # --- W[:, 3j:3j+3] = [1, alpha_j, beta_j]
    W = sp.tile([128, 24], F32, tag="W")
    nc.vector.memset(W[:], 1.0)
    for j in range(8):
        nc.vector.tensor_copy(W[:, 3 * j + 1 : 3 * j + 2], alpha[:, j : j + 1])
        nc.vector.tensor_copy(W[:, 3 * j + 2 : 3 * j + 3], beta[:, j : j + 1])

    # --- transposes -> [n-part, k-free] subtiles; S-side sums (contract n)
    psC = ps_cs.tile([128, 3], F32, tag="psC")
    psS = ps_cs.tile([128, 3], F32, tag="psS")
    for j in range(8):
        sl = slice(128 * j, 128 * (j + 1))
        tpc = ps_t.tile([128, 128], F32, tag="tp")
        nc.tensor.transpose(tpc[:], cmat[:, sl], ident[:])
        cnk = wp.tile([128, 128], F32, tag="cnk")
        nc.vector.tensor_copy(cnk[:], tpc[:])
        nc.tensor.matmul(psC[:], cnk[:], W[:, 3 * j : 3 * j + 3],
                         start=(j == 0), stop=(j == 7))
        tps = ps_t.tile([128, 128], F32, tag="tp")
        nc.tensor.transpose(tps[:], smat[:, sl], ident[:])
        snk = wp.tile([128, 128], F32, tag="snk")
        nc.vector.tensor_copy(snk[:], tps[:])
        nc.tensor.matmul(psS[:], snk[:], W[:, 3 * j : 3 * j + 3],
                         start=(j == 0), stop=(j == 7))

    # --- S_R, S_I  [128, 1]
    tmp1 = sp.tile([128, 1], F32, tag="tmp1")
    nc.vector.tensor_scalar(tmp1[:], psC[:, 1:2], cst[:, 1:2], None, ALU.mult)
    SR = sp.tile([128, 1], F32, tag="SR")
    nc.vector.scalar_tensor_tensor(SR[:], psC[:, 0:1], cst[:, 0:1], tmp1[:],
                                   ALU.mult, ALU.add)
    nc.vector.scalar_tensor_tensor(SR[:], psS[:, 2:3], cst[:, 2:3], SR[:],
                                   ALU.mult, ALU.add)
    tmp2 = sp.tile([128, 1], F32, tag="tmp2")
    nc.vector.tensor_scalar(tmp2[:], psS[:, 1:2], cst[:, 1:2], None, ALU.mult)
    SI = sp.tile([128, 1], F32, tag="SI")
    nc.vector.scalar_tensor_tensor(SI[:], psS[:, 0:1], cst[:, 0:1], tmp2[:],
                                   ALU.mult, ALU.add)
    nc.vector.scalar_tensor_tensor(SI[:], psC[:, 2:3], cst[:, 3:4], SI[:],
                                   ALU.mult, ALU.add)

    # --- U vectors [128, 6]; col order: [pw1*X, qw1*X, pw2*X, qw2*X, dw1*Y, dw2*Y]
    UC = sp.tile([128, 6], F32, tag="UC")
    US = sp.tile([128, 6], F32, tag="US")
    nc.vector.tensor_mul(UC[:, 0:4], cst[:, 4:8], _bc(SR, 4))
    nc.vector.tensor_mul(UC[:, 4:6], cst[:, 8:10], _bc(SI, 2))
    nc.vector.tensor_mul(US[:, 0:4], cst[:, 4:8], _bc(SI, 4))
    nc.vector.tensor_mul(US[:, 4:6], cst[:, 10:12], _bc(SR, 2))

    # --- T-side: out[n, 6] per subtile; regions of [128, 48] PSUMs
    pTC = ps_T.tile([128, 48], F32, tag="pTC")
    pTS = ps_T.tile([128, 48], F32, tag="pTS")
    for j in range(8):
        sl = slice(128 * j, 128 * (j + 1))
        nc.tensor.matmul(pTC[:, 6 * j : 6 * j + 6], cmat[:, sl], UC[:],
                         start=True, stop=True)
        nc.tensor.matmul(pTS[:, 6 * j : 6 * j + 6], smat[:, sl], US[:],
                         start=True, stop=True)

    # --- combine + self-energy + store
    # (TensorTensor may read at most one PSUM operand -> stage pTS in SBUF)
    sTS = sp.tile([128, 48], F32, tag="sTS")
    nc.vector.tensor_copy(sTS[:], pTS[:])
    aa = sp.tile([128, 8], F32, tag="aa")
    nc.vector.tensor_mul(aa[:], alpha[:], alpha[:])
    bb = sp.tile([128, 8], F32, tag="bb")
    nc.vector.tensor_mul(bb[:], beta[:], beta[:])

    for i, (cA, cB, cC, sc) in enumerate(((0, 1, 4, sc1), (2, 3, 5, sc2))):
        TT = sp.tile([128, 8], F32, tag="TT")
        nc.vector.tensor_add(TT[:], _st(pTC, cA), _st(sTS, cA))
        tb = sp.tile([128, 8], F32, tag="tb")
        nc.vector.tensor_add(tb[:], _st(pTC, cB), _st(sTS, cB))
        tb2 = sp.tile([128, 8], F32, tag="tb2")
        nc.vector.tensor_mul(tb2[:], tb[:], alpha[:])
        nc.vector.tensor_add(TT[:], TT[:], tb2[:])
        tcc = sp.tile([128, 8], F32, tag="tcc")
        nc.vector.tensor_add(tcc[:], _st(pTC, cC), _st(sTS, cC))
        tc2 = sp.tile([128, 8], F32, tag="tc2")
        nc.vector.tensor_mul(tc2[:], tcc[:], beta[:])
        nc.vector.tensor_add(TT[:], TT[:], tc2[:])

        sacc = sp.tile([128, 8], F32, tag="sacc")
        nc.vector.tensor_scalar(sacc[:], alpha[:], sc[1], sc[0], ALU.mult, ALU.add)
        nc.vector.scalar_tensor_tensor(sacc[:], aa[:], sc[2], sacc[:],
                                       ALU.mult, ALU.add)
        nc.vector.scalar_tensor_tensor(sacc[:], bb[:], sc[3], sacc[:],
                                       ALU.mult, ALU.add)
        ei = sp.tile([128, 8], F32, tag="ei")
        nc.vector.tensor_sub(ei[:], TT[:], sacc[:])
        nc.sync.dma_start(out_t[b].rearrange("(j p) c -> p j c", p=128)[:, :, i],
                          ei[:])


def _bc(col_ap, n):
    """Broadcast a [128, 1] tile AP along free dim to [128, n] (step 0)."""
    ap = col_ap[:]
    return bass.AP(ap.tensor, ap.offset, [ap.ap[0], [0, n]])


def _st(psum_tile, col):
    """Strided [128, 8] view of [128, 48] PSUM: cols col, col+6, ..."""
    ap = psum_tile[:]
    return bass.AP(ap.tensor, ap.offset + col, [ap.ap[0], [6, 8]])


def _build_program(sc1, sc2, debug=False):
    nc = bacc.Bacc(None, target_bir_lowering=False, debug=debug)
    t_in = nc.declare_dram_parameter("t", [BPC, N], F32, isOutput=False)
    kv_in = nc.declare_dram_parameter("kv", [1, KTRUNC], F32, isOutput=False)
    cst_in = nc.declare_dram_parameter("cst", [128, 12], F32, isOutput=False)
    id_in = nc.declare_dram_parameter("ident", [128, 128], F32, isOutput=False)
    out_t = nc.declare_dram_parameter("out", [BPC, N, 2], F32, isOutput=True)

    with tile.TileContext(nc) as tc:
        import contextlib
        with contextlib.ExitStack() as ctx:
            pc = ctx.enter_context(tc.tile_pool(name="const", bufs=1))
            wp = ctx.enter_context(tc.tile_pool(name="work", bufs=2))
            sp = ctx.enter_context(tc.tile_pool(name="small", bufs=2))
            ps_u = ctx.enter_context(tc.tile_pool(name="psu", bufs=1, space="PSUM"))
            ps_t = ctx.enter_context(tc.tile_pool(name="pst", bufs=2, space="PSUM"))
            ps_cs = ctx.enter_context(tc.tile_pool(name="pscs", bufs=1, space="PSUM"))
            ps_T = ctx.enter_context(tc.tile_pool(name="psT", bufs=1, space="PSUM"))

            ident = pc.tile([128, 128], F32, tag="ident")
            nc.sync.dma_start(ident[:], id_in[:])
            cst = pc.tile([128, 12], F32, tag="cst")
            nc.sync.dma_start(cst[:], cst_in[:])
            kv = pc.tile([1, KTRUNC], F32, tag="kv")
            nc.sync.dma_start(kv[:], kv_in[:])

            pools = (pc, wp, sp, ps_u, ps_t, ps_cs, ps_T)
            for b in range(BPC):
                _emit_batch(nc, tc, pools, b, t_in, out_t, kv, cst, ident,
                            sc1, sc2)
    return nc


def kernel(x, shift0, shift1, amp0, amp1):
    x = np.asarray(x, dtype=np.float32)
    cst, sc1, sc2 = _host_constants(shift0.reshape(-1)[0], shift1.reshape(-1)[0],
                                    amp0.reshape(-1)[0], amp1.reshape(-1)[0])
    nc = _build_program(sc1, sc2)
    nc.finalize()

    t_full = (x.astype(np.float64) / (2.0 * np.pi)).astype(np.float32)
    kvals = np.arange(KTRUNC, dtype=np.float32).reshape(1, KTRUNC)
    ident = np.eye(128, dtype=np.float32)
    in_maps = []
    for c in range(NCORES):
        in_maps.append({
            "t": t_full[BPC * c : BPC * (c + 1)],
            "kv": kvals,
            "cst": cst,
            "ident": ident,
        })
    res = run_bass_kernel_spmd(nc, in_maps, list(range(NCORES)))
    out = np.concatenate([res.results[c]["out"] for c in range(NCORES)], axis=0)
    return out.astype(np.float32)



# revision 2
# speedup vs baseline: 2.8678x; 2.8678x over previous
"""Trainium2 Bass kernel for nn_NUFFTLayerMultiChannelInitMixed.

Math: the reference's spread->FFT->filter->IFFT->energy pipeline collapses to
an analytic-spectrum bilinear form. The Gaussian spread is deconvolved exactly
by the deconv^2 filter, so with ghat_n(k) ~ e^{-i k x_n} (alias images carry
weight e^{-tau(M-k)^2} ~ 3e-5 -- negligible vs the 2e-2 gate):

  e_i[n] = sum_k G_i(k) [cos(k x_n) C(k) + sin(k x_n) S(k)] + off_i
  C(k) = sum_n cos(k x_n),  S(k) = sum_n sin(k x_n)
  G_i = pref * w * deconv2 * mult_i * p^2  (~1/k^2 decay; K=64 keeps ~1e-4)

Layout trick: with K=64, cos rows and sin rows stack into ONE [128, N] matrix.
The +1/4-turn cos shift rides in the phase matmul itself (contraction K=2:
[k; bias] x [t; 1]).  Per batch: 1 fp32 phase matmul, round/sub (DVE), one
Sin activation (bf16 out, accum_out = the n-row-sums for free), then 8 bf16
matmuls (stationary = trig chunk, rhs = G*[C;S] in [128,2]) yield the
energies directly in [n-part, channel] layout. ~37 PE instrs/core total.
Sharding: batch-parallel, 2 of 16 batches per core, no collectives.
"""

import numpy as np

try:
    import concourse.bass as bass
except ImportError:
    import sys
    sys.path.insert(0, "/opt/trn_rl_repo")
    import concourse.bass as bass

import concourse.bacc as bacc
import concourse.mybir as mybir
from concourse import tile
from concourse.bass_utils import run_bass_kernel_spmd

F32 = mybir.dt.float32
BF16 = mybir.dt.bfloat16
AF = mybir.ActivationFunctionType
ALU = mybir.AluOpType

M = 2001
L = 2.0 * np.pi
TAU = 12.0 * (L / (2.0 * np.pi * M)) ** 2
K = 64                   # spectral truncation (1/k^2 filter decay)
N = 1024
B_FULL = 16
NCORES = 8
BPC = B_FULL // NCORES   # batches per core
NT = BPC * N             # 2048 points handled per core in one sweep
MAGIC = 12582912.0       # 1.5 * 2^23: (u + MAGIC) - MAGIC = round-to-nearest(u)
PI = float(np.pi)


def _host_constants(shift0, shift1, amp0, amp1):
    """fp64 host-side k-space weights -> cst2 [128, 2], kvb [2, 128], offs."""
    k = np.arange(K, dtype=np.float64)
    tau = float(TAU)
    p2 = np.exp(-2.0 * tau * k * k)
    deconv2 = (np.pi / tau) * np.exp(2.0 * tau * k * k)
    mult1 = float(amp0) * (4.0 * np.pi) / (k * k + (1.0 * float(shift0)) ** 2)
    mult2 = float(amp1) * (4.0 * np.pi) / (k * k + (0.5 * float(shift1)) ** 2)
    w = np.full(K, 2.0)
    w[0] = 1.0
    Cc = (M / L) * np.sqrt(4.0 * np.pi * tau)
    scale = 1.0 / ((2.0 * np.pi * M / L) * (2.0 * np.pi))
    pref = scale * Cc * Cc / M
    G1 = pref * w * deconv2 * mult1 * p2
    G2 = pref * w * deconv2 * mult2 * p2

    cst2 = np.zeros((128, 2), dtype=np.float64)
    cst2[0:K, 0] = G1
    cst2[K:2 * K, 0] = G1
    cst2[0:K, 1] = G2
    cst2[K:2 * K, 1] = G2
    # k=0: cos term is the constant G[0]*N (folded into off), sin term is 0
    cst2[0, :] = 0.0
    cst2[K, :] = 0.0

    off1 = float(G1[0] * N - G1.sum())
    off2 = float(G2[0] * N - G2.sum())

    kvb = np.zeros((2, 128), dtype=np.float64)
    kvb[0, 0:K] = k
    kvb[0, K:2 * K] = k
    kvb[1, 0:K] = 0.25        # quarter-turn: sin(2pi(kt + 1/4)) = cos(2pi kt)
    return cst2.astype(np.float32), kvb.astype(np.float32), off1, off2


def _stv(tile_, start, step, num):
    """Strided [128, num] column view of a [128, *] tile."""
    ap = tile_[:]
    return bass.AP(ap.tensor, ap.offset + start, [ap.ap[0], [step, num]])


def _build_program(off1, off2, debug=False):
    nc = bacc.Bacc(None, target_bir_lowering=False, debug=debug)
    t_in = nc.declare_dram_parameter("t", [2, NT], F32, isOutput=False)
    kvb_in = nc.declare_dram_parameter("kvb", [2, 128], F32, isOutput=False)
    cst_in = nc.declare_dram_parameter("cst2", [128, 2], F32, isOutput=False)
    out_t = nc.declare_dram_parameter("out", [128, 16 * BPC], F32, isOutput=True)

    NQ = NT // 512  # 512-col quarters through the phase/trig pipeline

    with tile.TileContext(nc) as tc:
        import contextlib
        with contextlib.ExitStack() as ctx:
            pc = ctx.enter_context(tc.tile_pool(name="const", bufs=1))
            wp = ctx.enter_context(tc.tile_pool(name="work", bufs=NQ))
            sp = ctx.enter_context(tc.tile_pool(name="small", bufs=1))
            ps_u = ctx.enter_context(tc.tile_pool(name="psu", bufs=NQ, space="PSUM"))
            ps_T = ctx.enter_context(tc.tile_pool(name="psT", bufs=1, space="PSUM"))

            kvb = pc.tile([2, 128], F32, tag="kvb")
            nc.sync.dma_start(kvb[:], kvb_in[:])
            cst2 = pc.tile([128, 2], F32, tag="cst2")
            nc.sync.dma_start(cst2[:], cst_in[:])
            t_ext = pc.tile([2, NT], F32, tag="t")
            nc.sync.dma_start(t_ext[:], t_in[:])

            CS = sp.tile([128, NT], BF16, tag="CS")
            csum = sp.tile([128, NQ], F32, tag="csum")

            for q in range(NQ):
                sl = slice(512 * q, 512 * (q + 1))
                u = ps_u.tile([128, 512], F32, tag="u")
                nc.tensor.matmul(u[:], kvb[:], t_ext[:, sl], start=True, stop=True)
                rni = wp.tile([128, 512], F32, tag="rni")
                nc.vector.tensor_scalar(rni[:], u[:], MAGIC, MAGIC,
                                        ALU.add, ALU.subtract)
                r = wp.tile([128, 512], F32, tag="r")
                nc.vector.tensor_sub(r[:], u[:], rni[:])
                nc.scalar.activation(CS[:, sl], r[:], AF.Sin, scale=2.0 * PI,
                                     accum_out=csum[:, q:q + 1])

            pT = ps_T.tile([128, 16 * BPC], F32, tag="pT")
            QB = NQ // BPC  # quarters per batch
            for b in range(BPC):
                s = sp.tile([128, 1], F32, tag=f"s{b}")
                nc.vector.tensor_add(s[:], csum[:, QB * b:QB * b + 1],
                                     csum[:, QB * b + 1:QB * b + 2])
                UC = sp.tile([128, 2], BF16, tag=f"UC{b}")
                nc.vector.tensor_scalar(UC[:], cst2[:], s[:], None, ALU.mult)
                for j in range(8):
                    lh = CS[:, N * b + 128 * j: N * b + 128 * (j + 1)]
                    nc.tensor.matmul(pT[:, 16 * b + 2 * j: 16 * b + 2 * j + 2],
                                     lh, UC[:], start=True, stop=True)

            e = sp.tile([128, 16 * BPC], F32, tag="e")
            for b in range(BPC):
                for i, off in enumerate((off1, off2)):
                    nc.vector.tensor_scalar(_stv(e, 16 * b + i, 2, 8),
                                            _stv(pT, 16 * b + i, 2, 8),
                                            off, None, ALU.add)
            nc.sync.dma_start(out_t[:], e[:])
    return nc


def kernel(x, shift0, shift1, amp0, amp1):
    x = np.asarray(x, dtype=np.float32)
    cst2, kvb, off1, off2 = _host_constants(
        np.asarray(shift0).reshape(-1)[0], np.asarray(shift1).reshape(-1)[0],
        np.asarray(amp0).reshape(-1)[0], np.asarray(amp1).reshape(-1)[0])
    nc = _build_program(off1, off2)
    nc.finalize()

    t_full = (x.astype(np.float64) / (2.0 * np.pi)).astype(np.float32)
    in_maps = []
    for c in range(NCORES):
        t_ext = np.ones((2, NT), dtype=np.float32)
        t_ext[0] = t_full[BPC * c: BPC * (c + 1)].reshape(NT)
        in_maps.append({"t": t_ext, "kvb": kvb, "cst2": cst2})
    res = run_bass_kernel_spmd(nc, in_maps, list(range(NCORES)))
    outs = []
    for c in range(NCORES):
        arr = res.results[c]["out"]                      # [128, 16*BPC]
        arr = arr.reshape(128, BPC, 8, 2)                # (p, b, j, i)
        outs.append(arr.transpose(1, 2, 0, 3).reshape(BPC, N, 2))
    return np.concatenate(outs, axis=0).astype(np.float32)


# revision 4
# speedup vs baseline: 3.5266x; 1.2297x over previous
"""Trainium2 Bass kernel for nn_NUFFTLayerMultiChannelInitMixed.

Math: the reference's spread->FFT->filter->IFFT->energy pipeline collapses to
an analytic-spectrum bilinear form. The Gaussian spread is deconvolved exactly
by the deconv^2 filter, so with ghat_n(k) ~ e^{-i k x_n} (alias images carry
weight e^{-tau(M-k)^2} ~ 3e-5 -- negligible vs the 2e-2 gate):

  e_i[n] = sum_k G_i(k) [cos(k x_n) C(k) + sin(k x_n) S(k)] + off_i
  C(k) = sum_n cos(k x_n),  S(k) = sum_n sin(k x_n)
  G_i = pref * w * deconv2 * mult_i * p^2  (~1/k^2 decay; K=64 keeps ~1.5e-4)

Layout: with K=64, cos rows and sin rows stack into ONE [128, N] matrix; the
+1/4-turn cos shift and a 3-way bf16 split of t (k*t_hi + k*t_mid + k*t_lo
exact in fp32 PSUM) ride in a single K=4 bf16 phase matmul per 512 cols.
Range-reduce (round-to-nearest via +MAGIC, alternating ACT/DVE to balance
engines), one Sin activation (bf16 out, accum_out = row sums for free), then
8 bf16 matmuls per batch (stationary = trig chunk, rhs = G*[C;S] [128,2])
yield energies directly in [n-part, channel] layout.
Sharding: batch-parallel, 2 of 16 batches per core, no collectives.
"""

import numpy as np

try:
    import concourse.bass as bass
except ImportError:
    import sys
    sys.path.insert(0, "/opt/trn_rl_repo")
    import concourse.bass as bass

import concourse.bacc as bacc
import concourse.mybir as mybir
from concourse import tile
from concourse.bass_utils import run_bass_kernel_spmd

F32 = mybir.dt.float32
BF16 = mybir.dt.bfloat16
AF = mybir.ActivationFunctionType
ALU = mybir.AluOpType

M = 2001
L = 2.0 * np.pi
TAU = 12.0 * (L / (2.0 * np.pi * M)) ** 2
K = 64                   # spectral truncation (1/k^2 filter decay)
N = 1024
B_FULL = 16
NCORES = 8
BPC = B_FULL // NCORES   # batches per core
NT = BPC * N             # 2048 points handled per core in one sweep
MAGIC = 12582912.0       # 1.5 * 2^23: (u + MAGIC) - MAGIC = round-to-nearest(u)
PI = float(np.pi)


def _bf16(a):
    a32 = np.asarray(a, dtype=np.float32)
    u32 = a32.view(np.uint32).astype(np.uint64)
    return (((u32 + 0x7FFF + ((u32 >> 16) & 1)) & 0xFFFF0000)
            .astype(np.uint32)).view(np.float32)


def _host_constants(shift0, shift1, amp0, amp1):
    """fp64 host-side k-space weights -> cst2 [128, 2] and channel offsets."""
    k = np.arange(K, dtype=np.float64)
    tau = float(TAU)
    p2 = np.exp(-2.0 * tau * k * k)
    deconv2 = (np.pi / tau) * np.exp(2.0 * tau * k * k)
    mult1 = float(amp0) * (4.0 * np.pi) / (k * k + (1.0 * float(shift0)) ** 2)
    mult2 = float(amp1) * (4.0 * np.pi) / (k * k + (0.5 * float(shift1)) ** 2)
    w = np.full(K, 2.0)
    w[0] = 1.0
    Cc = (M / L) * np.sqrt(4.0 * np.pi * tau)
    scale = 1.0 / ((2.0 * np.pi * M / L) * (2.0 * np.pi))
    pref = scale * Cc * Cc / M
    G1 = pref * w * deconv2 * mult1 * p2
    G2 = pref * w * deconv2 * mult2 * p2

    cst2 = np.zeros((128, 2), dtype=np.float64)
    cst2[0:K, 0] = G1
    cst2[K:2 * K, 0] = G1
    cst2[0:K, 1] = G2
    cst2[K:2 * K, 1] = G2
    # k=0: cos term is the constant G[0]*N (folded into off), sin term is 0
    cst2[0, :] = 0.0
    cst2[K, :] = 0.0

    off1 = float(G1[0] * N - G1.sum())
    off2 = float(G2[0] * N - G2.sum())
    return cst2.astype(np.float32), off1, off2


def _pack_t(t_row):
    """[NT] fp32 t values -> [4, NT+128] bf16: 3-way split rows + ones row,
    with the phase-matmul stationary [k; k; k; bias] packed at cols NT:."""
    th = _bf16(t_row)
    tm = _bf16(t_row.astype(np.float64) - th.astype(np.float64))
    tl = _bf16(t_row.astype(np.float64) - th.astype(np.float64)
               - tm.astype(np.float64))
    ext = np.ones((4, NT + 128), dtype=np.float32)
    ext[0, :NT] = th
    ext[1, :NT] = tm
    ext[2, :NT] = tl
    kv = np.concatenate([np.arange(K), np.arange(K)]).astype(np.float32)
    ext[0, NT:] = kv
    ext[1, NT:] = kv
    ext[2, NT:] = kv
    ext[3, NT:] = np.where(np.arange(128) < K, 0.25, 0.0)
    import ml_dtypes
    return ext.astype(ml_dtypes.bfloat16)


def _stv(tile_, start, step, num):
    """Strided [128, num] column view of a [128, *] tile."""
    ap = tile_[:]
    return bass.AP(ap.tensor, ap.offset + start, [ap.ap[0], [step, num]])


def _build_program(off1, off2, debug=False):
    nc = bacc.Bacc(None, target_bir_lowering=False, debug=debug)
    t_in = nc.declare_dram_parameter("t", [4, NT + 128], BF16, isOutput=False)
    cst_in = nc.declare_dram_parameter("cst2", [128, 2], F32, isOutput=False)
    out_t = nc.declare_dram_parameter("out", [128, 16 * BPC], F32, isOutput=True)

    NQ = NT // 512  # 512-col quarters through the phase/trig pipeline

    with tile.TileContext(nc) as tc:
        import contextlib
        with contextlib.ExitStack() as ctx:
            pc = ctx.enter_context(tc.tile_pool(name="const", bufs=1))
            wp = ctx.enter_context(tc.tile_pool(name="work", bufs=NQ))
            sp = ctx.enter_context(tc.tile_pool(name="small", bufs=1))
            ps_u = ctx.enter_context(tc.tile_pool(name="psu", bufs=NQ, space="PSUM"))
            ps_T = ctx.enter_context(tc.tile_pool(name="psT", bufs=1, space="PSUM"))

            t_ext = pc.tile([4, NT + 128], BF16, tag="t")
            nc.sync.dma_start(t_ext[:], t_in[:])
            cst2 = pc.tile([128, 2], F32, tag="cst2")
            nc.sync.dma_start(cst2[:], cst_in[:])
            kvb = t_ext[:, NT:NT + 128]

            CS = sp.tile([128, NT], BF16, tag="CS")
            csum = sp.tile([128, NQ], F32, tag="csum")

            for q in range(NQ):
                sl = slice(512 * q, 512 * (q + 1))
                u = ps_u.tile([128, 512], F32, tag="u")
                nc.tensor.matmul(u[:], kvb, t_ext[:, sl], start=True, stop=True)
                if q % 2 == 0:
                    # ACT-led reduction: rniM = u + MAGIC (rounded), on ScalarE
                    rniM = wp.tile([128, 512], F32, tag="rniM")
                    nc.scalar.activation(rniM[:], u[:], AF.Copy, bias=MAGIC)
                    negr = wp.tile([128, 512], F32, tag="negr")
                    nc.vector.scalar_tensor_tensor(negr[:], rniM[:], MAGIC, u[:],
                                                   ALU.subtract, ALU.subtract)
                    nc.scalar.activation(CS[:, sl], negr[:], AF.Sin,
                                         scale=-2.0 * PI,
                                         accum_out=csum[:, q:q + 1])
                else:
                    # DVE-led reduction
                    rni = wp.tile([128, 512], F32, tag="rni")
                    nc.vector.tensor_scalar(rni[:], u[:], MAGIC, MAGIC,
                                            ALU.add, ALU.subtract)
                    r = wp.tile([128, 512], F32, tag="r")
                    nc.vector.tensor_sub(r[:], u[:], rni[:])
                    nc.scalar.activation(CS[:, sl], r[:], AF.Sin,
                                         scale=2.0 * PI,
                                         accum_out=csum[:, q:q + 1])

            pT = ps_T.tile([128, 16 * BPC], F32, tag="pT")
            e = sp.tile([128, 16 * BPC], F32, tag="e")
            QB = NQ // BPC  # quarters per batch
            for b in range(BPC):
                s = sp.tile([128, 1], F32, tag=f"s{b}")
                nc.gpsimd.tensor_add(s[:], csum[:, QB * b:QB * b + 1],
                                     csum[:, QB * b + 1:QB * b + 2])
                UC = sp.tile([128, 2], BF16, tag=f"UC{b}")
                nc.gpsimd.tensor_scalar(UC[:], cst2[:], s[:], None, ALU.mult)
                for j in range(8):
                    lh = CS[:, N * b + 128 * j: N * b + 128 * (j + 1)]
                    nc.tensor.matmul(pT[:, 16 * b + 2 * j: 16 * b + 2 * j + 2],
                                     lh, UC[:], start=True, stop=True)
                for i, off in enumerate((off1, off2)):
                    nc.vector.tensor_scalar(_stv(e, 16 * b + i, 2, 8),
                                            _stv(pT, 16 * b + i, 2, 8),
                                            off, None, ALU.add)
                nc.sync.dma_start(out_t[:, 16 * b:16 * (b + 1)],
                                  e[:, 16 * b:16 * (b + 1)])
    return nc


def kernel(x, shift0, shift1, amp0, amp1):
    x = np.asarray(x, dtype=np.float32)
    cst2, off1, off2 = _host_constants(
        np.asarray(shift0).reshape(-1)[0], np.asarray(shift1).reshape(-1)[0],
        np.asarray(amp0).reshape(-1)[0], np.asarray(amp1).reshape(-1)[0])
    nc = _build_program(off1, off2)
    nc.finalize()

    t_full = (x.astype(np.float64) / (2.0 * np.pi)).astype(np.float32)
    in_maps = []
    for c in range(NCORES):
        t_ext = _pack_t(t_full[BPC * c: BPC * (c + 1)].reshape(NT))
        in_maps.append({"t": t_ext, "cst2": cst2})
    res = run_bass_kernel_spmd(nc, in_maps, list(range(NCORES)))
    outs = []
    for c in range(NCORES):
        arr = res.results[c]["out"]                      # [128, 16*BPC]
        arr = arr.reshape(128, BPC, 8, 2)                # (p, b, j, i)
        outs.append(arr.transpose(1, 2, 0, 3).reshape(BPC, N, 2))
    return np.concatenate(outs, axis=0).astype(np.float32)


# revision 9
# speedup vs baseline: 3.8037x; 1.0786x over previous
"""Trainium2 Bass kernel for nn_NUFFTLayerMultiChannelInitMixed.

Math: the reference's spread->FFT->filter->IFFT->energy pipeline collapses to
an analytic-spectrum bilinear form. The Gaussian spread is deconvolved exactly
by the deconv^2 filter, so with ghat_n(k) ~ e^{-i k x_n} (alias images carry
weight e^{-tau(M-k)^2} ~ 3e-5 -- negligible vs the 2e-2 gate):

  e_i[n] = sum_k G_i(k) [cos(k x_n) C(k) + sin(k x_n) S(k)] + off_i
  C(k) = sum_n cos(k x_n),  S(k) = sum_n sin(k x_n)
  G_i = pref * w * deconv2 * mult_i * p^2  (~1/k^2 decay; K=64 keeps ~1.5e-4)

Layout: with K=64, cos rows and sin rows stack into ONE [128, N] matrix; the
+1/4-turn cos shift and a 3-way bf16 split of t (k*t_hi + k*t_mid + k*t_lo
exact in fp32 PSUM) ride in a single K=4 bf16 phase matmul per 512 cols.
Range-reduce (round-to-nearest via +MAGIC, alternating ACT/DVE to balance
engines), one Sin activation (bf16 out, accum_out = row sums for free), then
8 bf16 matmuls per batch (stationary = trig chunk, rhs = G*[C;S] [128,2])
yield energies directly in [n-part, channel] layout.
Sharding: batch-parallel, 2 of 16 batches per core, no collectives.
"""

import numpy as np

try:
    import concourse.bass as bass
except ImportError:
    import sys
    sys.path.insert(0, "/opt/trn_rl_repo")
    import concourse.bass as bass

import concourse.bacc as bacc
import concourse.mybir as mybir
from concourse import tile
from concourse.bass_utils import run_bass_kernel_spmd

F32 = mybir.dt.float32
BF16 = mybir.dt.bfloat16
AF = mybir.ActivationFunctionType
ALU = mybir.AluOpType

M = 2001
L = 2.0 * np.pi
TAU = 12.0 * (L / (2.0 * np.pi * M)) ** 2
K = 64                   # spectral truncation (1/k^2 filter decay)
N = 1024
B_FULL = 16
NCORES = 8
BPC = B_FULL // NCORES   # batches per core
NT = BPC * N             # 2048 points handled per core in one sweep
MAGIC = 12582912.0       # 1.5 * 2^23: (u + MAGIC) - MAGIC = round-to-nearest(u)
PI = float(np.pi)


def _bf16(a):
    a32 = np.asarray(a, dtype=np.float32)
    u32 = a32.view(np.uint32).astype(np.uint64)
    return (((u32 + 0x7FFF + ((u32 >> 16) & 1)) & 0xFFFF0000)
            .astype(np.uint32)).view(np.float32)


def _host_constants(shift0, shift1, amp0, amp1):
    """fp64 host-side k-space weights -> cst2 [128, 2] and channel offsets."""
    k = np.arange(K, dtype=np.float64)
    tau = float(TAU)
    p2 = np.exp(-2.0 * tau * k * k)
    deconv2 = (np.pi / tau) * np.exp(2.0 * tau * k * k)
    mult1 = float(amp0) * (4.0 * np.pi) / (k * k + (1.0 * float(shift0)) ** 2)
    mult2 = float(amp1) * (4.0 * np.pi) / (k * k + (0.5 * float(shift1)) ** 2)
    w = np.full(K, 2.0)
    w[0] = 1.0
    Cc = (M / L) * np.sqrt(4.0 * np.pi * tau)
    scale = 1.0 / ((2.0 * np.pi * M / L) * (2.0 * np.pi))
    pref = scale * Cc * Cc / M
    G1 = pref * w * deconv2 * mult1 * p2
    G2 = pref * w * deconv2 * mult2 * p2

    cst2 = np.zeros((128, 2), dtype=np.float64)
    cst2[0:K, 0] = G1
    cst2[K:2 * K, 0] = G1
    cst2[0:K, 1] = G2
    cst2[K:2 * K, 1] = G2
    # k=0: cos term is the constant G[0]*N (folded into off), sin term is 0
    cst2[0, :] = 0.0
    cst2[K, :] = 0.0

    off1 = float(G1[0] * N - G1.sum())
    off2 = float(G2[0] * N - G2.sum())
    return cst2.astype(np.float32), off1, off2


def _pack_t(t_row):
    """[NT] fp32 t values -> [4, 128+NT] bf16: the phase-matmul stationary
    [k; k; k; bias] at cols 0:128 (so it lands first in the DMA), then the
    3-way split rows of t + ones row."""
    th = _bf16(t_row)
    tm = _bf16(t_row.astype(np.float64) - th.astype(np.float64))
    tl = _bf16(t_row.astype(np.float64) - th.astype(np.float64)
               - tm.astype(np.float64))
    ext = np.ones((4, 128 + NT), dtype=np.float32)
    ext[0, 128:] = th
    ext[1, 128:] = tm
    ext[2, 128:] = tl
    kv = np.concatenate([np.arange(K), np.arange(K)]).astype(np.float32)
    ext[0, :128] = kv
    ext[1, :128] = kv
    ext[2, :128] = kv
    ext[3, :128] = np.where(np.arange(128) < K, 0.25, 0.0)
    import ml_dtypes
    return ext.astype(ml_dtypes.bfloat16)


def _stv(tile_, start, step, num):
    """Strided [128, num] column view of a [128, *] tile."""
    ap = tile_[:]
    return bass.AP(ap.tensor, ap.offset + start, [ap.ap[0], [step, num]])


def _build_program(off1, off2, debug=False):
    nc = bacc.Bacc(None, target_bir_lowering=False, debug=debug)
    t_in = nc.declare_dram_parameter("t", [4, 128 + NT], BF16, isOutput=False)
    cst_in = nc.declare_dram_parameter("cst2", [128, 2], F32, isOutput=False)
    out_t = nc.declare_dram_parameter("out", [128, 16 * BPC], F32, isOutput=True)

    NQ = NT // 512  # 512-col quarters through the phase/trig pipeline

    with tile.TileContext(nc) as tc:
        import contextlib
        with contextlib.ExitStack() as ctx:
            pc = ctx.enter_context(tc.tile_pool(name="const", bufs=1))
            wp = ctx.enter_context(tc.tile_pool(name="work", bufs=NQ))
            sp = ctx.enter_context(tc.tile_pool(name="small", bufs=1))
            ps_u = ctx.enter_context(tc.tile_pool(name="psu", bufs=NQ, space="PSUM"))
            ps_T = ctx.enter_context(tc.tile_pool(name="psT", bufs=1, space="PSUM"))

            # Dummy Sin on scratch: makes the FIRST ScalarE op a Sin so the
            # compiler resident-set pick contains sin (its sets also contain
            # identity), avoiding a 1.3us mid-pipeline ACT_TABLE_LOAD swap.
            dummy = sp.tile([1, 2], F32, tag="dummy")
            nc.vector.memset(dummy[:], 0.0)
            dummy2 = sp.tile([1, 2], F32, tag="dummy2")
            nc.scalar.activation(dummy2[:], dummy[:], AF.Sin, scale=1.0)
            magicc = pc.tile([128, 1], F32, tag="magic")
            nc.gpsimd.memset(magicc[:], MAGIC)

            t_ext = pc.tile([4, 128 + NT], BF16, tag="t")
            nc.sync.dma_start(t_ext[:, 0:128 + NT // 2], t_in[:, 0:128 + NT // 2])
            nc.sync.dma_start(t_ext[:, 128 + NT // 2:], t_in[:, 128 + NT // 2:])
            cst2 = pc.tile([128, 2], F32, tag="cst2")
            nc.sync.dma_start(cst2[:], cst_in[:])
            kvb = t_ext[:, 0:128]

            CS = sp.tile([128, NT], BF16, tag="CS")
            csum = sp.tile([128, NQ], F32, tag="csum")

            for q in range(NQ):
                sl = slice(512 * q, 512 * (q + 1))
                tsl = slice(128 + 512 * q, 128 + 512 * (q + 1))
                u = ps_u.tile([128, 512], F32, tag="u")
                nc.tensor.matmul(u[:], kvb, t_ext[:, tsl], start=True, stop=True)
                if q % 2 == 0:
                    # ACT-led reduction: rniM = u + MAGIC (rounded), on ScalarE
                    rniM = wp.tile([128, 512], F32, tag="rniM")
                    nc.scalar.activation(rniM[:], u[:], AF.Identity,
                                         bias=magicc[:])
                    negr = wp.tile([128, 512], F32, tag="negr")
                    nc.vector.scalar_tensor_tensor(negr[:], rniM[:], MAGIC, u[:],
                                                   ALU.subtract, ALU.subtract)
                    nc.scalar.activation(CS[:, sl], negr[:], AF.Sin,
                                         scale=-2.0 * PI,
                                         accum_out=csum[:, q:q + 1])
                else:
                    # DVE-led reduction
                    rni = wp.tile([128, 512], F32, tag="rni")
                    nc.vector.tensor_scalar(rni[:], u[:], MAGIC, MAGIC,
                                            ALU.add, ALU.subtract)
                    r = wp.tile([128, 512], F32, tag="r")
                    nc.vector.tensor_sub(r[:], u[:], rni[:])
                    nc.scalar.activation(CS[:, sl], r[:], AF.Sin,
                                         scale=2.0 * PI,
                                         accum_out=csum[:, q:q + 1])

            pT = ps_T.tile([128, 16 * BPC], F32, tag="pT")
            e = sp.tile([128, 16 * BPC], F32, tag="e")
            QB = NQ // BPC  # quarters per batch
            for b in range(BPC):
                s = sp.tile([128, 1], F32, tag=f"s{b}")
                nc.gpsimd.tensor_add(s[:], csum[:, QB * b:QB * b + 1],
                                     csum[:, QB * b + 1:QB * b + 2])
                UC = sp.tile([128, 2], BF16, tag=f"UC{b}")
                nc.gpsimd.tensor_scalar(UC[:], cst2[:], s[:], None, ALU.mult)
                for j in range(8):
                    lh = CS[:, N * b + 128 * j: N * b + 128 * (j + 1)]
                    nc.tensor.matmul(pT[:, 16 * b + 2 * j: 16 * b + 2 * j + 2],
                                     lh, UC[:], start=True, stop=True)
                for i, off in enumerate((off1, off2)):
                    nc.vector.tensor_scalar(_stv(e, 16 * b + i, 2, 8),
                                            _stv(pT, 16 * b + i, 2, 8),
                                            off, None, ALU.add)
                nc.sync.dma_start(out_t[:, 16 * b:16 * (b + 1)],
                                  e[:, 16 * b:16 * (b + 1)])
    return nc


def kernel(x, shift0, shift1, amp0, amp1):
    x = np.asarray(x, dtype=np.float32)
    cst2, off1, off2 = _host_constants(
        np.asarray(shift0).reshape(-1)[0], np.asarray(shift1).reshape(-1)[0],
        np.asarray(amp0).reshape(-1)[0], np.asarray(amp1).reshape(-1)[0])
    nc = _build_program(off1, off2)
    nc.finalize()

    t_full = (x.astype(np.float64) / (2.0 * np.pi)).astype(np.float32)
    in_maps = []
    for c in range(NCORES):
        t_ext = _pack_t(t_full[BPC * c: BPC * (c + 1)].reshape(NT))
        in_maps.append({"t": t_ext, "cst2": cst2})
    res = run_bass_kernel_spmd(nc, in_maps, list(range(NCORES)))
    outs = []
    for c in range(NCORES):
        arr = res.results[c]["out"]                      # [128, 16*BPC]
        arr = arr.reshape(128, BPC, 8, 2)                # (p, b, j, i)
        outs.append(arr.transpose(1, 2, 0, 3).reshape(BPC, N, 2))
    return np.concatenate(outs, axis=0).astype(np.float32)


# revision 12
# speedup vs baseline: 3.8729x; 1.0182x over previous
"""Trainium2 Bass kernel for nn_NUFFTLayerMultiChannelInitMixed.

Math: the reference's spread->FFT->filter->IFFT->energy pipeline collapses to
an analytic-spectrum bilinear form. The Gaussian spread is deconvolved exactly
by the deconv^2 filter, so with ghat_n(k) ~ e^{-i k x_n} (alias images carry
weight e^{-tau(M-k)^2} ~ 3e-5 -- negligible vs the 2e-2 gate):

  e_i[n] = sum_k G_i(k) [cos(k x_n) C(k) + sin(k x_n) S(k)] + off_i
  C(k) = sum_n cos(k x_n),  S(k) = sum_n sin(k x_n)
  G_i = pref * w * deconv2 * mult_i * p^2  (~1/k^2 decay; K=64 keeps ~1.5e-4)

Layout: with K=64, cos rows and sin rows stack into ONE [128, N] matrix; the
+1/4-turn cos shift and a 3-way bf16 split of t (k*t_hi + k*t_mid + k*t_lo
exact in fp32 PSUM) ride in a single K=4 bf16 phase matmul per 512 cols.
Range-reduce (round-to-nearest via +MAGIC, alternating ACT/DVE to balance
engines), one Sin activation (bf16 out, accum_out = row sums for free), then
8 bf16 matmuls per batch (stationary = trig chunk, rhs = G*[C;S] [128,2])
yield energies directly in [n-part, channel] layout.
Sharding: batch-parallel, 2 of 16 batches per core, no collectives.
"""

import numpy as np

try:
    import concourse.bass as bass
except ImportError:
    import sys
    sys.path.insert(0, "/opt/trn_rl_repo")
    import concourse.bass as bass

import concourse.bacc as bacc
import concourse.mybir as mybir
from concourse import tile
from concourse.bass_utils import run_bass_kernel_spmd

F32 = mybir.dt.float32
BF16 = mybir.dt.bfloat16
AF = mybir.ActivationFunctionType
ALU = mybir.AluOpType

M = 2001
L = 2.0 * np.pi
TAU = 12.0 * (L / (2.0 * np.pi * M)) ** 2
K = 64                   # spectral truncation (1/k^2 filter decay)
N = 1024
B_FULL = 16
NCORES = 8
BPC = B_FULL // NCORES   # batches per core
NT = BPC * N             # 2048 points handled per core in one sweep
MAGIC = 12582912.0       # 1.5 * 2^23: (u + MAGIC) - MAGIC = round-to-nearest(u)
PI = float(np.pi)


def _bf16(a):
    a32 = np.asarray(a, dtype=np.float32)
    u32 = a32.view(np.uint32).astype(np.uint64)
    return (((u32 + 0x7FFF + ((u32 >> 16) & 1)) & 0xFFFF0000)
            .astype(np.uint32)).view(np.float32)


def _host_constants(shift0, shift1, amp0, amp1):
    """fp64 host-side k-space weights -> cst2 [128, 2] and channel offsets."""
    k = np.arange(K, dtype=np.float64)
    tau = float(TAU)
    p2 = np.exp(-2.0 * tau * k * k)
    deconv2 = (np.pi / tau) * np.exp(2.0 * tau * k * k)
    mult1 = float(amp0) * (4.0 * np.pi) / (k * k + (1.0 * float(shift0)) ** 2)
    mult2 = float(amp1) * (4.0 * np.pi) / (k * k + (0.5 * float(shift1)) ** 2)
    w = np.full(K, 2.0)
    w[0] = 1.0
    Cc = (M / L) * np.sqrt(4.0 * np.pi * tau)
    scale = 1.0 / ((2.0 * np.pi * M / L) * (2.0 * np.pi))
    pref = scale * Cc * Cc / M
    G1 = pref * w * deconv2 * mult1 * p2
    G2 = pref * w * deconv2 * mult2 * p2

    cst2 = np.zeros((128, 2), dtype=np.float64)
    cst2[0:K, 0] = G1
    cst2[K:2 * K, 0] = G1
    cst2[0:K, 1] = G2
    cst2[K:2 * K, 1] = G2
    # k=0: cos term is the constant G[0]*N (folded into off), sin term is 0
    cst2[0, :] = 0.0
    cst2[K, :] = 0.0

    off1 = float(G1[0] * N - G1.sum())
    off2 = float(G2[0] * N - G2.sum())
    return cst2.astype(np.float32), off1, off2


def _pack_t(t_row):
    """[NT] fp32 t values -> [4, 128+NT] bf16: the phase-matmul stationary
    [k; k; k; bias] at cols 0:128 (so it lands first in the DMA), then the
    3-way split rows of t + ones row."""
    th = _bf16(t_row)
    tm = _bf16(t_row.astype(np.float64) - th.astype(np.float64))
    tl = _bf16(t_row.astype(np.float64) - th.astype(np.float64)
               - tm.astype(np.float64))
    ext = np.ones((4, 128 + NT), dtype=np.float32)
    ext[0, 128:] = th
    ext[1, 128:] = tm
    ext[2, 128:] = tl
    kv = np.concatenate([np.arange(K), np.arange(K)]).astype(np.float32)
    ext[0, :128] = kv
    ext[1, :128] = kv
    ext[2, :128] = kv
    ext[3, :128] = np.where(np.arange(128) < K, 0.25, 0.0)
    import ml_dtypes
    return ext.astype(ml_dtypes.bfloat16)


def _stv(tile_, start, step, num):
    """Strided [128, num] column view of a [128, *] tile."""
    ap = tile_[:]
    return bass.AP(ap.tensor, ap.offset + start, [ap.ap[0], [step, num]])


def _build_program(off1, off2, debug=False):
    nc = bacc.Bacc(None, target_bir_lowering=False, debug=debug)
    t_in = nc.declare_dram_parameter("t", [4, 128 + NT], BF16, isOutput=False)
    cst_in = nc.declare_dram_parameter("cst2", [128, 2], F32, isOutput=False)
    out_t = nc.declare_dram_parameter("out", [128, 16 * BPC], F32, isOutput=True)

    NQ = NT // 512  # 512-col quarters through the phase/trig pipeline

    with tile.TileContext(nc) as tc:
        import contextlib
        with contextlib.ExitStack() as ctx:
            pc = ctx.enter_context(tc.tile_pool(name="const", bufs=1))
            wp = ctx.enter_context(tc.tile_pool(name="work", bufs=NQ))
            sp = ctx.enter_context(tc.tile_pool(name="small", bufs=1))
            ps_u = ctx.enter_context(tc.tile_pool(name="psu", bufs=NQ, space="PSUM"))
            ps_T = ctx.enter_context(tc.tile_pool(name="psT", bufs=1, space="PSUM"))

            # Dummy Sin on scratch: makes the FIRST ScalarE op a Sin so the
            # compiler resident-set pick contains sin (its sets also contain
            # identity), avoiding a 1.3us mid-pipeline ACT_TABLE_LOAD swap.
            dummy = sp.tile([1, 2], F32, tag="dummy")
            nc.vector.memset(dummy[:], 0.0)
            dummy2 = sp.tile([1, 2], F32, tag="dummy2")
            nc.scalar.activation(dummy2[:], dummy[:], AF.Sin, scale=1.0)
            magicc = pc.tile([128, 1], F32, tag="magic")
            nc.gpsimd.memset(magicc[:], MAGIC)
            offs = pc.tile([128, 16], F32, tag="offs")
            nc.gpsimd.memset(_stv(offs, 0, 2, 8), off1)
            nc.gpsimd.memset(_stv(offs, 1, 2, 8), off2)

            t_ext = pc.tile([4, 128 + NT], BF16, tag="t")
            nc.sync.dma_start(t_ext[:, 0:128 + NT // 2], t_in[:, 0:128 + NT // 2])
            nc.sync.dma_start(t_ext[:, 128 + NT // 2:], t_in[:, 128 + NT // 2:])
            cst2 = pc.tile([128, 2], F32, tag="cst2")
            nc.sync.dma_start(cst2[:], cst_in[:])
            kvb = t_ext[:, 0:128]

            CS = sp.tile([128, NT], BF16, tag="CS")
            csum = sp.tile([128, NQ], F32, tag="csum")

            for q in range(NQ):
                sl = slice(512 * q, 512 * (q + 1))
                tsl = slice(128 + 512 * q, 128 + 512 * (q + 1))
                u = ps_u.tile([128, 512], F32, tag="u")
                nc.tensor.matmul(u[:], kvb, t_ext[:, tsl], start=True, stop=True)
                if q % 2 == 0:
                    # ACT-led reduction: rniM = u + MAGIC (rounded), on ScalarE
                    rniM = wp.tile([128, 512], F32, tag="rniM")
                    nc.scalar.activation(rniM[:], u[:], AF.Identity,
                                         bias=magicc[:])
                    negr = wp.tile([128, 512], F32, tag="negr")
                    nc.vector.scalar_tensor_tensor(negr[:], rniM[:], MAGIC, u[:],
                                                   ALU.subtract, ALU.subtract)
                    nc.scalar.activation(CS[:, sl], negr[:], AF.Sin,
                                         scale=-2.0 * PI,
                                         accum_out=csum[:, q:q + 1])
                else:
                    # DVE-led reduction
                    rni = wp.tile([128, 512], F32, tag="rni")
                    nc.vector.tensor_scalar(rni[:], u[:], MAGIC, MAGIC,
                                            ALU.add, ALU.subtract)
                    r = wp.tile([128, 512], F32, tag="r")
                    nc.vector.tensor_sub(r[:], u[:], rni[:])
                    nc.scalar.activation(CS[:, sl], r[:], AF.Sin,
                                         scale=2.0 * PI,
                                         accum_out=csum[:, q:q + 1])

            pT = ps_T.tile([128, 16 * BPC], F32, tag="pT")
            e = sp.tile([128, 16 * BPC], F32, tag="e")
            QB = NQ // BPC  # quarters per batch
            for b in range(BPC):
                # UC = cst2 * (csum_q0 + csum_q1), staged so only the stt is
                # on the critical path after the batch's last accum lands
                UCa = sp.tile([128, 2], F32, tag=f"UCa{b}")
                nc.gpsimd.tensor_scalar(UCa[:], cst2[:],
                                        csum[:, QB * b:QB * b + 1],
                                        None, ALU.mult)
                UC = sp.tile([128, 2], BF16, tag=f"UC{b}")
                nc.vector.scalar_tensor_tensor(
                    UC[:], cst2[:], csum[:, QB * b + 1:QB * b + 2], UCa[:],
                    ALU.mult, ALU.add)
                for j in range(8):
                    lh = CS[:, N * b + 128 * j: N * b + 128 * (j + 1)]
                    nc.tensor.matmul(pT[:, 16 * b + 2 * j: 16 * b + 2 * j + 2],
                                     lh, UC[:], start=True, stop=True)
                nc.vector.tensor_add(e[:, 16 * b:16 * (b + 1)],
                                     pT[:, 16 * b:16 * (b + 1)],
                                     offs[:, 0:16])
                nc.sync.dma_start(out_t[:, 16 * b:16 * (b + 1)],
                                  e[:, 16 * b:16 * (b + 1)])
    return nc


def kernel(x, shift0, shift1, amp0, amp1):
    x = np.asarray(x, dtype=np.float32)
    cst2, off1, off2 = _host_constants(
        np.asarray(shift0).reshape(-1)[0], np.asarray(shift1).reshape(-1)[0],
        np.asarray(amp0).reshape(-1)[0], np.asarray(amp1).reshape(-1)[0])
    nc = _build_program(off1, off2)
    nc.finalize()

    t_full = (x.astype(np.float64) / (2.0 * np.pi)).astype(np.float32)
    in_maps = []
    for c in range(NCORES):
        t_ext = _pack_t(t_full[BPC * c: BPC * (c + 1)].reshape(NT))
        in_maps.append({"t": t_ext, "cst2": cst2})
    res = run_bass_kernel_spmd(nc, in_maps, list(range(NCORES)))
    outs = []
    for c in range(NCORES):
        arr = res.results[c]["out"]                      # [128, 16*BPC]
        arr = arr.reshape(128, BPC, 8, 2)                # (p, b, j, i)
        outs.append(arr.transpose(1, 2, 0, 3).reshape(BPC, N, 2))
    return np.concatenate(outs, axis=0).astype(np.float32)


# revision 17
# speedup vs baseline: 3.8837x; 1.0028x over previous
"""Trainium2 Bass kernel for nn_NUFFTLayerMultiChannelInitMixed.

Math: the reference's spread->FFT->filter->IFFT->energy pipeline collapses to
an analytic-spectrum bilinear form. The Gaussian spread is deconvolved exactly
by the deconv^2 filter, so with ghat_n(k) ~ e^{-i k x_n} (alias images carry
weight e^{-tau(M-k)^2} ~ 3e-5 -- negligible vs the 2e-2 gate):

  e_i[n] = sum_k G_i(k) [cos(k x_n) C(k) + sin(k x_n) S(k)] + off_i
  C(k) = sum_n cos(k x_n),  S(k) = sum_n sin(k x_n)
  G_i = pref * w * deconv2 * mult_i * p^2  (~1/k^2 decay; K=64 keeps ~1.5e-4)

Layout: with K=64, cos rows and sin rows stack into ONE [128, N] matrix; the
+1/4-turn cos shift and a 3-way bf16 split of t (k*t_hi + k*t_mid + k*t_lo
exact in fp32 PSUM) ride in a single K=4 bf16 phase matmul per 512 cols.
Range-reduce (round-to-nearest via +MAGIC, alternating ACT/DVE to balance
engines), one Sin activation (bf16 out, accum_out = row sums for free), then
8 bf16 matmuls per batch (stationary = trig chunk, rhs = G*[C;S] [128,2])
yield energies directly in [n-part, channel] layout.
Sharding: batch-parallel, 2 of 16 batches per core, no collectives.
"""

import numpy as np

try:
    import concourse.bass as bass
except ImportError:
    import sys
    sys.path.insert(0, "/opt/trn_rl_repo")
    import concourse.bass as bass

import concourse.bacc as bacc
import concourse.mybir as mybir
from concourse import tile
from concourse.bass_utils import run_bass_kernel_spmd

F32 = mybir.dt.float32
BF16 = mybir.dt.bfloat16
AF = mybir.ActivationFunctionType
ALU = mybir.AluOpType

M = 2001
L = 2.0 * np.pi
TAU = 12.0 * (L / (2.0 * np.pi * M)) ** 2
K = 64                   # spectral truncation (1/k^2 filter decay)
N = 1024
B_FULL = 16
NCORES = 8
BPC = B_FULL // NCORES   # batches per core
NT = BPC * N             # 2048 points handled per core in one sweep
MAGIC = 12582912.0       # 1.5 * 2^23: (u + MAGIC) - MAGIC = round-to-nearest(u)
PI = float(np.pi)


def _bf16(a):
    a32 = np.asarray(a, dtype=np.float32)
    u32 = a32.view(np.uint32).astype(np.uint64)
    return (((u32 + 0x7FFF + ((u32 >> 16) & 1)) & 0xFFFF0000)
            .astype(np.uint32)).view(np.float32)


def _host_constants(shift0, shift1, amp0, amp1):
    """fp64 host-side k-space weights -> cst2 [128, 2] and channel offsets."""
    k = np.arange(K, dtype=np.float64)
    tau = float(TAU)
    p2 = np.exp(-2.0 * tau * k * k)
    deconv2 = (np.pi / tau) * np.exp(2.0 * tau * k * k)
    mult1 = float(amp0) * (4.0 * np.pi) / (k * k + (1.0 * float(shift0)) ** 2)
    mult2 = float(amp1) * (4.0 * np.pi) / (k * k + (0.5 * float(shift1)) ** 2)
    w = np.full(K, 2.0)
    w[0] = 1.0
    Cc = (M / L) * np.sqrt(4.0 * np.pi * tau)
    scale = 1.0 / ((2.0 * np.pi * M / L) * (2.0 * np.pi))
    pref = scale * Cc * Cc / M
    G1 = pref * w * deconv2 * mult1 * p2
    G2 = pref * w * deconv2 * mult2 * p2

    cst2 = np.zeros((128, 2), dtype=np.float64)
    cst2[0:K, 0] = G1
    cst2[K:2 * K, 0] = G1
    cst2[0:K, 1] = G2
    cst2[K:2 * K, 1] = G2

    # k=0 rows: cos row (0) is identically 1, and the sin k=0 row (64) is
    # made identically 1 too by giving it the +0.25 bias. Ride the constant
    # per-channel offset off_i = G_i[0]*N - sum(G_i) through them: row 0
    # carries the bf16-representable part, row 64 the residual, so no
    # precision is lost to UC's bf16 and no separate offset-add is needed.
    off1 = float(G1[0] * N - G1.sum())
    off2 = float(G2[0] * N - G2.sum())
    for i, off in enumerate((off1, off2)):
        hi = float(_bf16(np.float32(off)))
        cst2[0, i] = hi / N
        cst2[K, i] = (off - hi) / N
    return cst2.astype(np.float32)


def _pack_t(t_row):
    """[NT] fp32 t values -> [4, 128+NT] bf16: the phase-matmul stationary
    [k; k; k; bias] at cols 0:128 (so it lands first in the DMA), then the
    3-way split rows of t + ones row."""
    th = _bf16(t_row)
    tm = _bf16(t_row.astype(np.float64) - th.astype(np.float64))
    tl = _bf16(t_row.astype(np.float64) - th.astype(np.float64)
               - tm.astype(np.float64))
    ext = np.ones((4, 128 + NT), dtype=np.float32)
    ext[0, 128:] = th
    ext[1, 128:] = tm
    ext[2, 128:] = tl
    kv = np.concatenate([np.arange(K), np.arange(K)]).astype(np.float32)
    ext[0, :128] = kv
    ext[1, :128] = kv
    ext[2, :128] = kv
    bias = np.where(np.arange(128) < K, 0.25, 0.0)
    bias[K] = 0.25            # sin k=0 row -> constant 1, carries the offset
    ext[3, :128] = bias
    import ml_dtypes
    return ext.astype(ml_dtypes.bfloat16)


def _stv(tile_, start, step, num):
    """Strided [128, num] column view of a [128, *] tile."""
    ap = tile_[:]
    return bass.AP(ap.tensor, ap.offset + start, [ap.ap[0], [step, num]])


def _build_program(debug=False):
    nc = bacc.Bacc(None, target_bir_lowering=False, debug=debug)
    t_in = nc.declare_dram_parameter("t", [4, 128 + NT], BF16, isOutput=False)
    cst_in = nc.declare_dram_parameter("cst2", [128, 2], F32, isOutput=False)
    out_t = nc.declare_dram_parameter("out", [128, 16 * BPC], F32, isOutput=True)

    NQ = NT // 512  # 512-col quarters through the phase/trig pipeline

    with tile.TileContext(nc) as tc:
        import contextlib
        with contextlib.ExitStack() as ctx:
            pc = ctx.enter_context(tc.tile_pool(name="const", bufs=1))
            wp = ctx.enter_context(tc.tile_pool(name="work", bufs=NQ))
            sp = ctx.enter_context(tc.tile_pool(name="small", bufs=1))
            ps_u = ctx.enter_context(tc.tile_pool(name="psu", bufs=NQ, space="PSUM"))
            ps_T = ctx.enter_context(tc.tile_pool(name="psT", bufs=1, space="PSUM"))

            # Dummy Sin on scratch: makes the FIRST ScalarE op a Sin so the
            # compiler resident-set pick contains sin (its sets also contain
            # identity), avoiding a 1.3us mid-pipeline ACT_TABLE_LOAD swap.
            dummy = sp.tile([1, 2], F32, tag="dummy")
            nc.vector.memset(dummy[:], 0.0)
            dummy2 = sp.tile([1, 2], F32, tag="dummy2")
            nc.scalar.activation(dummy2[:], dummy[:], AF.Sin, scale=1.0)
            magicc = pc.tile([128, 1], F32, tag="magic")
            nc.gpsimd.memset(magicc[:], MAGIC)

            t_ext = pc.tile([4, 128 + NT], BF16, tag="t")
            nc.sync.dma_start(t_ext[:, 0:128 + NT // 2], t_in[:, 0:128 + NT // 2])
            nc.sync.dma_start(t_ext[:, 128 + NT // 2:], t_in[:, 128 + NT // 2:])
            cst2 = pc.tile([128, 2], F32, tag="cst2")
            nc.sync.dma_start(cst2[:], cst_in[:])
            kvb = t_ext[:, 0:128]

            CS = sp.tile([128, NT], BF16, tag="CS")
            csum = sp.tile([128, NQ], F32, tag="csum")

            for q in range(NQ):
                sl = slice(512 * q, 512 * (q + 1))
                tsl = slice(128 + 512 * q, 128 + 512 * (q + 1))
                u = ps_u.tile([128, 512], F32, tag="u")
                nc.tensor.matmul(u[:], kvb, t_ext[:, tsl], start=True, stop=True)
                if q % 2 == 0:
                    # ACT-led reduction: rniM = u + MAGIC (rounded), on ScalarE
                    rniM = wp.tile([128, 512], F32, tag="rniM")
                    nc.scalar.activation(rniM[:], u[:], AF.Identity,
                                         bias=magicc[:])
                    negr = wp.tile([128, 512], F32, tag="negr")
                    nc.vector.scalar_tensor_tensor(negr[:], rniM[:], MAGIC, u[:],
                                                   ALU.subtract, ALU.subtract)
                    nc.scalar.activation(CS[:, sl], negr[:], AF.Sin,
                                         scale=-2.0 * PI,
                                         accum_out=csum[:, q:q + 1])
                else:
                    # DVE-led reduction
                    rni = wp.tile([128, 512], F32, tag="rni")
                    nc.vector.tensor_scalar(rni[:], u[:], MAGIC, MAGIC,
                                            ALU.add, ALU.subtract)
                    r = wp.tile([128, 512], F32, tag="r")
                    nc.vector.tensor_sub(r[:], u[:], rni[:])
                    nc.scalar.activation(CS[:, sl], r[:], AF.Sin,
                                         scale=2.0 * PI,
                                         accum_out=csum[:, q:q + 1])

            pT = ps_T.tile([128, 16 * BPC], F32, tag="pT")
            e = sp.tile([128, 16 * BPC], F32, tag="e")
            QB = NQ // BPC  # quarters per batch
            for b in range(BPC):
                # UC = cst2 * (csum_q0 + csum_q1), staged so only the stt is
                # on the critical path after the batch's last accum lands
                UCa = sp.tile([128, 2], F32, tag=f"UCa{b}")
                nc.gpsimd.tensor_scalar(UCa[:], cst2[:],
                                        csum[:, QB * b:QB * b + 1],
                                        None, ALU.mult)
                UC = sp.tile([128, 2], BF16, tag=f"UC{b}")
                nc.vector.scalar_tensor_tensor(
                    UC[:], cst2[:], csum[:, QB * b + 1:QB * b + 2], UCa[:],
                    ALU.mult, ALU.add)
                for j in range(8):
                    lh = CS[:, N * b + 128 * j: N * b + 128 * (j + 1)]
                    nc.tensor.matmul(pT[:, 16 * b + 2 * j: 16 * b + 2 * j + 2],
                                     lh, UC[:], start=True, stop=True)
                nc.vector.tensor_copy(e[:, 16 * b:16 * (b + 1)],
                                      pT[:, 16 * b:16 * (b + 1)])
                nc.sync.dma_start(out_t[:, 16 * b:16 * (b + 1)],
                                  e[:, 16 * b:16 * (b + 1)])
    return nc


def kernel(x, shift0, shift1, amp0, amp1):
    x = np.asarray(x, dtype=np.float32)
    cst2 = _host_constants(
        np.asarray(shift0).reshape(-1)[0], np.asarray(shift1).reshape(-1)[0],
        np.asarray(amp0).reshape(-1)[0], np.asarray(amp1).reshape(-1)[0])
    nc = _build_program()
    nc.finalize()

    t_full = (x.astype(np.float64) / (2.0 * np.pi)).astype(np.float32)
    in_maps = []
    for c in range(NCORES):
        t_ext = _pack_t(t_full[BPC * c: BPC * (c + 1)].reshape(NT))
        in_maps.append({"t": t_ext, "cst2": cst2})
    res = run_bass_kernel_spmd(nc, in_maps, list(range(NCORES)))
    outs = []
    for c in range(NCORES):
        arr = res.results[c]["out"]                      # [128, 16*BPC]
        arr = arr.reshape(128, BPC, 8, 2)                # (p, b, j, i)
        outs.append(arr.transpose(1, 2, 0, 3).reshape(BPC, N, 2))
    return np.concatenate(outs, axis=0).astype(np.float32)
